# revision 32
# baseline (speedup 1.0000x reference)
"""Trainium2 Bass kernel for nn_ChannelWisePatchLevelObfuscator.

Math: split each (512,512) image into 32x32 patches of 16x16; per (channel,
group) apply a dense 256->256 obfuscation matmul over patch pixels (group =
(row+col) % 32), add bias, tanh, then permute channels.

Sharding: over the 96 (channel, group) pairs -- 12 pairs per core, each pair
covering the FULL batch (64 images x 32 patches = 2048 matmul rows). Unlike
batch-parallel sharding (which replicates the 12 MiB fp16 weight tensor into
every core), this loads each weight exactly once somewhere.

Traffic: x in fp16 (12 MiB), weights fp16 (1.5 MiB), output quantized to
uint8 (6 MiB): tanh outputs live in [-1,1], so an 8-bit code costs ~3.6e-3
relative error against a 2e-2 budget and halves store traffic. Total
19.5 MiB/core vs 36 MiB for the fp16 batch-parallel baseline. The channel
permutation is applied for free while scattering per-core results into the
full output.

Schedule: the DMA flows ride different issuing engines/queues so nothing
head-of-line blocks and no busy engine pays HWDGE descriptor-gen time
(~0.7us per dma_start). Bias on the SWDGE (gpsimd) queue (the first
ACTIVATE needs it early; as a late rider on a busy ring it stalls the
activation train); all 24 x half-slabs dispatched up front on the SP ring
(all tiles resident, no pool-reuse throttling) with the 24 uint8 stores
BEHIND them on the same ring, so loads keep full bandwidth and stores drain
in their shadow; weights (4 small chunks) on the otherwise-idle ACT ring.
Dummy matmuls during the DMA preamble burn the HAM power limiter's ~3.4us
half-clock window so the real first matmuls (which gate the serial
activation train) run at full clock. The scalar engine does nothing but its
24 big ACTIVATEs -- the serial bottleneck (~48us of tanh at 1
elem/cycle/lane + 352-cycle fixed cost per ACTIVATE).

Per pair: per output half oc, K=2x128 accumulates into a 4-bank [128,2048]
PSUM tile (kc-outer so the 4 start-matmuls need only the first half-slab),
one ScalarE ACTIVATE does bias + tanh -> fp16 scratch, DVE applies
127*x + 128.5 with a round-to-nearest uint8 cast (host decodes with -128.5
to cancel the extra half-LSB), and a 256 KiB store per oc half streams the
result out.
"""
import sys
import numpy as np

sys.path.insert(0, "/opt/trn_rl_repo")

import concourse.bacc as bacc  # noqa: E402
import concourse.mybir as mybir  # noqa: E402
import concourse.tile as tile  # noqa: E402
from concourse.bass_utils import run_bass_kernel_spmd  # noqa: E402

IMG, C, PS, G, B = 512, 3, 16, 32, 64
NH = NW = IMG // PS          # 32 patches per side
P2 = PS * PS                 # 256 pixels per patch
NCORES = 8
NPAIR = C * G                # 96 (channel, group) pairs
PPC = NPAIR // NCORES        # 12 pairs per core
T = B * NH                   # 2048 matmul rows per pair: t = b*32 + r
NB = 4                       # N-blocks of 512 per oc half
NWC = 4                      # weight chunks per core
PWC = PPC // NWC             # pairs per weight chunk
QSCALE = 127.0               # uint8 quantization scale for tanh in [-1,1]

F32 = mybir.dt.float32
MM_DT = mybir.dt.float16     # matmul input dtype
ACT_DT = mybir.dt.float16    # activation output scratch dtype
OUT_DT = mybir.dt.uint8      # device store dtype; host dequantizes
NP_MM = np.float16

_g = np.arange(G)[:, None]
_r = np.arange(NH)[None, :]
COLS = (_g - _r) % NW        # (g, r) -> patch column belonging to group g

_CACHE = {}


def _build_nc():
    nc = bacc.Bacc("TRN2", target_bir_lowering=False, debug=False,
                   num_devices=NCORES)
    # xt[pair, kc, k_lo, t]: contraction p = kc*128 + k_lo on partitions;
    # each (pair, kc) half is one contiguous 512 KiB slab (4 KiB/partition),
    # so the first matmuls are gated by half a pair, not a full one.
    xt = nc.dram_tensor("xt", [PPC, 2, 128, T], MM_DT, kind="ExternalInput")
    # w[chunk, k_lo, (pair_in_chunk)*512 + kc*256 + o]: 384 KiB slabs.
    w = nc.dram_tensor("w", [NWC, 128, PWC * 2 * P2], MM_DT,
                       kind="ExternalInput")
    bias = nc.dram_tensor("bias", [128, PPC * 2], F32, kind="ExternalInput")
    # out[pair, oc, o_lo, t] uint8: oc-major so per-oc stores are contiguous
    out = nc.dram_tensor("out", [PPC, 2, 128, T], OUT_DT,
                         kind="ExternalOutput")

    with tile.TileContext(nc) as tc:
        with tc.tile_pool(name="biasp", bufs=1) as bias_pool, \
             tc.tile_pool(name="wp", bufs=NWC) as w_pool, \
             tc.tile_pool(name="xtp", bufs=2 * PPC) as xt_pool, \
             tc.tile_pool(name="sctp", bufs=4) as sc_pool, \
             tc.tile_pool(name="outp", bufs=2 * PPC) as out_pool, \
             tc.tile_pool(name="warmp", bufs=1) as warm_pool, \
             tc.tile_pool(name="psp", bufs=2, space="PSUM") as ps_pool:
            # bias via the SWDGE (gpsimd) queue: tiny, lands early, and
            # keeps both HWDGE rings clear for their first big riders
            bias_sb = bias_pool.tile([128, PPC * 2], F32)
            nc.gpsimd.dma_start(bias_sb[:], bias[:, :])
            # PE warm-up: the HAM power limiter runs the first ~3.4us of PE
            # activity at half clock. Burn that window on dummy matmuls
            # during the DMA preamble so the real first matmuls (which gate
            # the serial activation train) run at full clock.
            warm_sb = warm_pool.tile([128, 512], MM_DT)
            nc.vector.memset(warm_sb[:], 0)
            # same pool tag as the real tiles so it shares their buffers
            ps = ps_pool.tile([128, NB * 512], F32)
            for i in range(8):
                nc.tensor.matmul(ps[:128, (i % NB) * 512:
                                    (i % NB) * 512 + 512],
                                 warm_sb[:, 0:128], warm_sb[:],
                                 start=True, stop=True)
            # weights on the ACT ring (otherwise idle)
            w_ts = []
            for ch in range(NWC):
                w_t = w_pool.tile([128, PWC * 2 * P2], MM_DT)
                nc.scalar.dma_start(w_t[:], w[ch])
                w_ts.append(w_t)
            # all x loads up front on the SP ring: every tile resident, so
            # load streaming is never throttled by pool reuse; stores ride
            # the same ring BEHIND every load packet, so loads keep full
            # bandwidth and the small store stream drains in their shadow
            xt_ts = []
            for pr in range(PPC):
                for kc in range(2):
                    xt_t = xt_pool.tile([128, T], MM_DT)
                    nc.sync.dma_start(xt_t[:], xt[pr, kc])
                    xt_ts.append(xt_t)
            for pr in range(PPC):
                w_sb = w_ts[pr // PWC]
                wb = (pr % PWC) * 2 * P2
                for oc in range(2):
                    ps = ps_pool.tile([128, NB * 512], F32)
                    # kc outer: the 4 start-matmuls need only the kc=0 half
                    # of the pair's x slab
                    for kc in range(2):
                        for nb in range(NB):
                            nc.tensor.matmul(
                                ps[:, nb * 512:(nb + 1) * 512],
                                w_sb[:, wb + kc * P2 + oc * 128:
                                     wb + kc * P2 + oc * 128 + 128],
                                xt_ts[pr * 2 + kc][:, nb * 512:
                                                   (nb + 1) * 512],
                                start=(kc == 0), stop=(kc == 1))
                    bidx = pr * 2 + oc
                    sc_t = sc_pool.tile([128, T], ACT_DT)
                    nc.scalar.activation(
                        sc_t[:], ps[:],
                        mybir.ActivationFunctionType.Tanh,
                        bias=bias_sb[:, bidx: bidx + 1],
                        scale=1.0)
                    # DVE quantize: trunc(127*x + 128.5) == round(127x)+128
                    out_t = out_pool.tile([128, T], OUT_DT)
                    nc.vector.tensor_scalar(
                        out_t[:], sc_t[:],
                        QSCALE, 128.5,
                        mybir.AluOpType.mult, mybir.AluOpType.add)
                    nc.sync.dma_start(out[pr, oc], out_t[:])
    nc.compile()
    return nc


def _pack_inputs(x, w_full, bias_full):
    # x (B, C, 512, 512) fp32 -> per-core xt[pair, kc, k_lo, t] slabs
    xp = x.astype(NP_MM).reshape(B, C, NH, PS, NW, PS)  # b c r py cl px
    sel = xp[:, :, _r, :, COLS, :]                      # g r b c py px
    xt = sel.transpose(3, 0, 4, 5, 2, 1).reshape(NPAIR, P2, T)
    xt = xt.reshape(NPAIR, 2, 128, T)
    xts = [np.ascontiguousarray(xt[m * PPC:(m + 1) * PPC])
           for m in range(NCORES)]

    # w [c, g, p_in, o] -> per-core [chunk, k_lo, pair*512 + kc*256 + o]
    w2 = w_full.astype(NP_MM).reshape(NPAIR, 2, 128, P2)
    ws = []
    for m in range(NCORES):
        sl = w2[m * PPC:(m + 1) * PPC].reshape(NWC, PWC, 2, 128, P2)
        ws.append(np.ascontiguousarray(
            sl.transpose(0, 3, 1, 2, 4).reshape(NWC, 128, PWC * 2 * P2)))

    # bias [c, g, o] -> [o_lo, pair*2 + oc]
    b2 = bias_full.reshape(NPAIR, 2, 128)
    bs = []
    for m in range(NCORES):
        sl = b2[m * PPC:(m + 1) * PPC].transpose(2, 0, 1)
        bs.append(np.ascontiguousarray(sl.reshape(128, PPC * 2)))
    return xts, ws, bs


def _unpack_out(results, perm):
    # results[m]["out"]: [12, 2(oc), 128(o_lo), 2048(b*32 + r)] uint8
    od = np.concatenate([results[m]["out"] for m in range(NCORES)])
    od = od.reshape(C, G, 2, 8, PS, B, NH)             # c g oc py_lo px b r
    src = od.transpose(1, 6, 5, 0, 2, 3, 4)            # g r b c oc py_lo px
    src = src.reshape(G, NH, B, C, PS, PS)             # py = oc*8 + py_lo
    tmp = np.empty((NH, NW, B, C, PS, PS), dtype=np.uint8)
    tmp[_r, COLS] = src                                # tmp[r, (g-r)%32]
    img = tmp.transpose(2, 3, 0, 4, 1, 5).reshape(B, C, IMG, IMG)
    img = img[:, perm].astype(np.float32)
    # decode assuming the device cast rounds-to-nearest ON TOP of the +0.5
    # in the affine (measured: -128.0 decode leaves a +0.5 LSB bias that
    # exactly doubles the error)
    img -= 128.5
    img *= 1.0 / QSCALE
    np.clip(img, -1.0, 1.0, out=img)
    return img


def kernel(x, obfuscation_weights, obfuscation_biases, channel_permutation):
    x = np.ascontiguousarray(x, dtype=np.float32)
    w = np.ascontiguousarray(obfuscation_weights, dtype=np.float32)
    bias = np.asarray(obfuscation_biases, dtype=np.float32)
    perm = np.asarray(channel_permutation, dtype=np.int64)

    if "nc" not in _CACHE:
        _CACHE["nc"] = _build_nc()
    nc = _CACHE["nc"]

    xts, ws, bs = _pack_inputs(x, w, bias)
    in_maps = [{"xt": xts[m], "w": ws[m], "bias": bs[m]}
               for m in range(NCORES)]

    res = run_bass_kernel_spmd(nc, in_maps, core_ids=list(range(NCORES)))
    _CACHE["last_results"] = res

    return _unpack_out(res.results, perm)


# revision 43
# speedup vs baseline: 1.2001x; 1.2001x over previous
"""Trainium2 Bass kernel for nn_ChannelWisePatchLevelObfuscator.

Math: split each (512,512) image into 32x32 patches of 16x16; per (channel,
group) apply a dense 256->256 obfuscation matmul over patch pixels (group =
(row+col) % 32), add bias, tanh, then permute channels.

Sharding: over the 96 (channel, group) pairs -- 12 pairs per core, each pair
covering the FULL batch (64 images x 32 patches = 2048 matmul rows). Unlike
batch-parallel sharding (which replicates the 12 MiB fp16 weight tensor into
every core), this loads each weight exactly once somewhere.

Traffic: x in fp16 (12 MiB), weights fp16 (1.5 MiB), output quantized to
uint8 (6 MiB): tanh outputs live in [-1,1], so an 8-bit code costs ~3.6e-3
relative error against a 2e-2 budget and halves store traffic. Total
19.5 MiB/core vs 36 MiB for the fp16 batch-parallel baseline. The channel
permutation is applied for free while scattering per-core results into the
full output.

Schedule: the DMA flows ride different issuing engines/queues so nothing
head-of-line blocks and no busy engine pays HWDGE descriptor-gen time
(~0.7us per dma_start). Bias on the SWDGE (gpsimd) queue (the first
ACTIVATE needs it early; as a late rider on a busy ring it stalls the
activation train); all 24 x half-slabs dispatched up front on the SP ring
(all tiles resident, no pool-reuse throttling) with the 24 uint8 stores
BEHIND them on the same ring, so loads keep full bandwidth and stores drain
in their shadow; weights (4 small chunks) on the otherwise-idle ACT ring.
Dummy matmuls during the DMA preamble burn the HAM power limiter's ~3.4us
half-clock window so the real first matmuls (which gate the serial
activation train) run at full clock. The scalar engine does nothing but its
24 big ACTIVATEs -- the serial bottleneck (~48us of tanh at 1
elem/cycle/lane + 352-cycle fixed cost per ACTIVATE).

Per pair: per output half oc, K=2x128 accumulates into a 4-bank [128,2048]
PSUM tile (kc-outer so the 4 start-matmuls need only the first half-slab),
one ScalarE ACTIVATE does bias + tanh -> fp16 scratch, DVE applies
127*x + 128.5 with a round-to-nearest uint8 cast (host decodes with -128.5
to cancel the extra half-LSB), and a 256 KiB store per oc half streams the
result out.
"""
import sys
import numpy as np

sys.path.insert(0, "/opt/trn_rl_repo")

import concourse.bacc as bacc  # noqa: E402
import concourse.mybir as mybir  # noqa: E402
import concourse.tile as tile  # noqa: E402
from concourse.bass_utils import run_bass_kernel_spmd  # noqa: E402

IMG, C, PS, G, B = 512, 3, 16, 32, 64
NH = NW = IMG // PS          # 32 patches per side
P2 = PS * PS                 # 256 pixels per patch
NCORES = 8
NPAIR = C * G                # 96 (channel, group) pairs
PPC = NPAIR // NCORES        # 12 pairs per core
T = B * NH                   # 2048 matmul rows per pair: t = b*32 + r
NB = 4                       # N-blocks of 512 per oc half
NWC = 4                      # weight chunks per core
PWC = PPC // NWC             # pairs per weight chunk
QSCALE = 127.0               # uint8 quantization scale for tanh in [-1,1]

F32 = mybir.dt.float32
MM_DT = mybir.dt.float16     # matmul input dtype
ACT_DT = mybir.dt.float16    # activation output scratch dtype
OUT_DT = mybir.dt.uint8      # device store dtype; host dequantizes
NP_MM = np.float16

_g = np.arange(G)[:, None]
_r = np.arange(NH)[None, :]
COLS = (_g - _r) % NW        # (g, r) -> patch column belonging to group g

_CACHE = {}


def _build_nc():
    nc = bacc.Bacc("TRN2", target_bir_lowering=False, debug=False,
                   num_devices=NCORES)
    # xt[pair, kc, k_lo, t]: contraction p = kc*128 + k_lo on partitions;
    # each (pair, kc) half is one contiguous 512 KiB slab (4 KiB/partition),
    # so the first matmuls are gated by half a pair, not a full one.
    xt = nc.dram_tensor("xt", [PPC, 2, 128, T], MM_DT, kind="ExternalInput")
    # w[chunk, k_lo, (pair_in_chunk)*512 + kc*256 + o]: 384 KiB slabs.
    # Chunk 0 carries 24 extra columns holding the bias in fp16 (its own
    # 12 KiB DMA takes 10-17us to land on any ring -- tiny descriptors
    # round-robin against MiB-scale packets -- and gated the first
    # ACTIVATE; riding inside w0 it lands at ~9.5us for free).
    w = nc.dram_tensor("w", [NWC, 128, PWC * 2 * P2 + PPC * 2], MM_DT,
                       kind="ExternalInput")
    # out[pair, oc, o_lo, t] uint8: oc-major so per-oc stores are contiguous
    out = nc.dram_tensor("out", [PPC, 2, 128, T], OUT_DT,
                         kind="ExternalOutput")

    with tile.TileContext(nc) as tc:
        with tc.tile_pool(name="biasp", bufs=1) as bias_pool, \
             tc.tile_pool(name="wp", bufs=NWC) as w_pool, \
             tc.tile_pool(name="xtp", bufs=2 * PPC) as xt_pool, \
             tc.tile_pool(name="sctp", bufs=4) as sc_pool, \
             tc.tile_pool(name="outp", bufs=2 * PPC) as out_pool, \
             tc.tile_pool(name="warmp", bufs=1) as warm_pool, \
             tc.tile_pool(name="psp", bufs=2, space="PSUM") as ps_pool:
            # PE warm-up: the HAM power limiter runs the first ~3.4us of PE
            # activity at half clock. Burn that window on dummy matmuls
            # during the DMA preamble so the real first matmuls (which gate
            # the serial activation train) run at full clock.
            warm_sb = warm_pool.tile([128, 512], MM_DT)
            nc.vector.memset(warm_sb[:], 0)
            # 128.5 column for the final scalar-engine quantization bias
            qoff_sb = warm_pool.tile([128, 1], F32)
            nc.vector.memset(qoff_sb[:], 128.5)
            # same pool tag as the real tiles so it shares their buffers
            ps = ps_pool.tile([128, NB * 512], F32)
            for i in range(6):
                nc.tensor.matmul(ps[:128, (i % NB) * 512:
                                    (i % NB) * 512 + 512],
                                 warm_sb[:, 0:128], warm_sb[:],
                                 start=True, stop=True)
            # weights on the ACT ring (otherwise idle)
            w_ts = []
            for ch in range(NWC):
                w_t = w_pool.tile([128, PWC * 2 * P2 + PPC * 2], MM_DT)
                nc.scalar.dma_start(w_t[:], w[ch])
                w_ts.append(w_t)
            bias_sb = w_ts[0]
            WOFF = PWC * 2 * P2
            # all x loads up front on the SP ring: every tile resident, so
            # load streaming is never throttled by pool reuse; stores ride
            # the same ring BEHIND every load packet, so loads keep full
            # bandwidth and the small store stream drains in their shadow
            xt_ts = []
            for pr in range(PPC):
                for kc in range(2):
                    xt_t = xt_pool.tile([128, T], MM_DT)
                    nc.sync.dma_start(xt_t[:], xt[pr, kc])
                    xt_ts.append(xt_t)
            for pr in range(PPC):
                w_sb = w_ts[pr // PWC]
                wb = (pr % PWC) * 2 * P2
                for oc in range(2):
                    first = (pr == 0 and oc == 0)
                    last = (pr == PPC - 1 and oc == 1)
                    ps = ps_pool.tile([128, NB * 512], F32)
                    # kc outer: the 4 start-matmuls need only the kc=0 half
                    # of the pair's x slab. For the very first tile, nb
                    # outer instead: banks complete after 2 matmuls each so
                    # the activation train starts ~1.5us earlier.
                    loops = ([(kc, nb) for nb in range(NB) for kc in range(2)]
                             if first else
                             [(kc, nb) for kc in range(2) for nb in range(NB)])
                    for kc, nb in loops:
                        nc.tensor.matmul(
                            ps[:, nb * 512:(nb + 1) * 512],
                            w_sb[:, wb + kc * P2 + oc * 128:
                                 wb + kc * P2 + oc * 128 + 128],
                            xt_ts[pr * 2 + kc][:, nb * 512:
                                               (nb + 1) * 512],
                            start=(kc == 0), stop=(kc == 1))
                    bidx = pr * 2 + oc
                    out_t = out_pool.tile([128, T], OUT_DT)
                    if first:
                        # two N=1024 activations: the first needs only the
                        # first two banks (4 matmuls)
                        for hf in range(2):
                            sc_t = sc_pool.tile([128, T // 2], ACT_DT)
                            nc.scalar.activation(
                                sc_t[:], ps[:, hf * 1024:(hf + 1) * 1024],
                                mybir.ActivationFunctionType.Tanh,
                                bias=bias_sb[:, WOFF + bidx: WOFF + bidx + 1],
                                scale=1.0)
                            nc.vector.tensor_scalar(
                                out_t[:, hf * 1024:(hf + 1) * 1024], sc_t[:],
                                QSCALE, 128.5,
                                mybir.AluOpType.mult, mybir.AluOpType.add)
                        nc.sync.dma_start(out[pr, oc], out_t[:])
                        continue
                    sc_t = sc_pool.tile([128, T], ACT_DT)
                    nc.scalar.activation(
                        sc_t[:], ps[:],
                        mybir.ActivationFunctionType.Tanh,
                        bias=bias_sb[:, WOFF + bidx: WOFF + bidx + 1],
                        scale=1.0)
                    if last:
                        # keep the whole final chain on the scalar engine:
                        # the last ACT's completion semaphore is held back
                        # by the engine's ~2us end-of-program DRAIN, so a
                        # cross-engine DVE handoff here pays ~2.4us. A
                        # program-order quant + store dispatch on scalar
                        # (draining on the idle ACT ring) avoids it.
                        nc.scalar.activation(
                            out_t[:], sc_t[:],
                            mybir.ActivationFunctionType.Identity,
                            bias=qoff_sb[:, 0:1], scale=QSCALE)
                        nc.scalar.dma_start(out[pr, oc], out_t[:])
                        continue
                    # DVE quantize: trunc(127*x + 128.5) == round(127x)+128
                    nc.vector.tensor_scalar(
                        out_t[:], sc_t[:],
                        QSCALE, 128.5,
                        mybir.AluOpType.mult, mybir.AluOpType.add)
                    nc.sync.dma_start(out[pr, oc], out_t[:])
    nc.compile()
    return nc


def _pack_inputs(x, w_full, bias_full):
    # x (B, C, 512, 512) fp32 -> per-core xt[pair, kc, k_lo, t] slabs
    xp = x.astype(NP_MM).reshape(B, C, NH, PS, NW, PS)  # b c r py cl px
    sel = xp[:, :, _r, :, COLS, :]                      # g r b c py px
    xt = sel.transpose(3, 0, 4, 5, 2, 1).reshape(NPAIR, P2, T)
    xt = xt.reshape(NPAIR, 2, 128, T)
    xts = [np.ascontiguousarray(xt[m * PPC:(m + 1) * PPC])
           for m in range(NCORES)]

    # w [c, g, p_in, o] -> per-core [chunk, k_lo, pair*512 + kc*256 + o],
    # with the fp16 bias block [o_lo, pair*2 + oc] riding in chunk 0's
    # 24 extra tail columns
    w2 = w_full.astype(NP_MM).reshape(NPAIR, 2, 128, P2)
    b2 = bias_full.reshape(NPAIR, 2, 128)
    ws = []
    for m in range(NCORES):
        sl = w2[m * PPC:(m + 1) * PPC].reshape(NWC, PWC, 2, 128, P2)
        wk = np.zeros((NWC, 128, PWC * 2 * P2 + PPC * 2), dtype=NP_MM)
        wk[:, :, :PWC * 2 * P2] = (
            sl.transpose(0, 3, 1, 2, 4).reshape(NWC, 128, PWC * 2 * P2))
        bsl = b2[m * PPC:(m + 1) * PPC].transpose(2, 0, 1)
        wk[0, :, PWC * 2 * P2:] = bsl.reshape(128, PPC * 2).astype(NP_MM)
        ws.append(wk)
    return xts, ws


def _unpack_out(results, perm):
    # results[m]["out"]: [12, 2(oc), 128(o_lo), 2048(b*32 + r)] uint8
    od = np.concatenate([results[m]["out"] for m in range(NCORES)])
    od = od.reshape(C, G, 2, 8, PS, B, NH)             # c g oc py_lo px b r
    src = od.transpose(1, 6, 5, 0, 2, 3, 4)            # g r b c oc py_lo px
    src = src.reshape(G, NH, B, C, PS, PS)             # py = oc*8 + py_lo
    tmp = np.empty((NH, NW, B, C, PS, PS), dtype=np.uint8)
    tmp[_r, COLS] = src                                # tmp[r, (g-r)%32]
    img = tmp.transpose(2, 3, 0, 4, 1, 5).reshape(B, C, IMG, IMG)
    img = img[:, perm].astype(np.float32)
    # decode assuming the device cast rounds-to-nearest ON TOP of the +0.5
    # in the affine (measured: -128.0 decode leaves a +0.5 LSB bias that
    # exactly doubles the error)
    img -= 128.5
    img *= 1.0 / QSCALE
    np.clip(img, -1.0, 1.0, out=img)
    return img


def kernel(x, obfuscation_weights, obfuscation_biases, channel_permutation):
    x = np.ascontiguousarray(x, dtype=np.float32)
    w = np.ascontiguousarray(obfuscation_weights, dtype=np.float32)
    bias = np.asarray(obfuscation_biases, dtype=np.float32)
    perm = np.asarray(channel_permutation, dtype=np.int64)

    if "nc" not in _CACHE:
        _CACHE["nc"] = _build_nc()
    nc = _CACHE["nc"]

    xts, ws = _pack_inputs(x, w, bias)
    in_maps = [{"xt": xts[m], "w": ws[m]} for m in range(NCORES)]

    res = run_bass_kernel_spmd(nc, in_maps, core_ids=list(range(NCORES)))
    _CACHE["last_results"] = res

    return _unpack_out(res.results, perm)
